# revision 31
# baseline (speedup 1.0000x reference)
"""Spatial self-attention scores kernel for Trainium2 (8 NeuronCores).

Computes, per batch b:
    qk = W @ x_b          # [256, 4096] = [256,256] @ [256,4096]
    q, k = qk[:128], qk[128:]
    sim = (q.T @ k) * 128**-0.5
    out_b = softmax(sim, axis=-1)        # [4096, 4096]
Output: [8, 1, 4096, 4096] float32.

Sharding: data-parallel over batch, one batch image per NeuronCore.

The kernel is ScalarE-bound: softmax's exp runs only on the scalar
engine (1 elem/cycle/lane @ 1.2 GHz => ~109 us body + ~27 us of
instruction/accumulator-read overhead for the 16.7M outputs per
core), so every other phase is arranged to hide under it:
  - x arrives as fp32 via three HWDGE DMAs on the Activation ring
    (ScalarE is idle pre-ramp, and this keeps x off the output ring)
    and is cast fp32->fp16 on DVE in 512-col pieces emitted at their
    point of use (up-front emission head-of-line blocks the in-order
    DVE queue on late x chunks). The ~12 us x transfer time is the
    startup floor.
  - fp16 projection matmuls -> q,k in SBUF as [d=128, s=4096] fp16,
    interleaved with the attention groups; PE warm-up matmuls keep
    the HAM clock ramping while x lands.
  - per 128-query row-tile: fp16 matmuls (K=128, N=512) into 4-bank
    PSUM tiles; one ScalarE ACTIVATE per 2048 columns computes
    exp(SCALE*sim) straight to fp16 with a fused row-sum (accum_out;
    measured cheaper than any DVE-side reduction); DVE combines the
    partial sums, takes the reciprocal, scales the row.
  - the first four row-tiles run column-major (each 512/1536/2048
    wave starts as soon as the x slice it needs has landed), so the
    in-order ACTIVATE queue never waits on the x DMA.
  - query rows interleave across output-group tiles (tile t of group
    g = rows g*512 + 4p + t): each partition then holds 4 consecutive
    DRAM rows = one contiguous 32 KB descriptor per partition per
    4 MB group DMA. Small descriptors otherwise cap the write drain
    at ~250 GB/s (vs ~340 measured with large ones).
  - output leaves as fp16 and is upcast to fp32 on the host (rel-err
    cost ~5e-4 against a 2e-2 budget): halves the HBM write traffic
    that roofline-bound the fp32 version at ~187 us/core; the last
    group ships per normalized half-row to shorten the tail.
"""

import numpy as np
from contextlib import ExitStack

import concourse.bass as bass
import concourse.tile as tile
from concourse import bacc, mybir
from concourse.bass_utils import run_bass_kernel_spmd
from concourse.masks import make_identity

B = 8
C = 256
HW = 4096
D = 128
SCALE = D ** -0.5
N_CORES = 8

BANK = 512             # PSUM bank width (fp32) = one matmul free-dim
ACT_CHUNK = 2048       # one ScalarE activation spans 4 banks
N_ACT = HW // ACT_CHUNK          # 2
GRP = 4                # row-tiles per output DMA (4 -> 4 MB fp16 transfers)
N_GRP = HW // (128 * GRP)        # 8
OUT_BUFS = 4

F32 = mybir.dt.float32
F16 = mybir.dt.float16
MM_DT = mybir.dt.float16
PROJ_DT = mybir.dt.float16

# x input DMA chunks (fp16, HWDGE on the Activation ring). x ships
# from the host already cast to fp16: the kernel used to cast
# fp32->fp16 on DVE before any use, so numerics are identical, but
# the input load halves to 2.1 MB (~6.5 us at the ~330 GB/s per-core
# read path -- splitting across two queues was measured useless, the
# queues just share the same path at half rate each). The first 512
# columns land alone so the first attention wave can start.
X_DMA = ((0, 512), (512, 2048), (2048, 4096))


def _emit(ctx: ExitStack, tc: tile.TileContext, out_ap, x_ap, w_ap):
    nc = tc.nc

    const = ctx.enter_context(tc.tile_pool(name="const", bufs=1))
    data = ctx.enter_context(tc.tile_pool(name="data", bufs=1))
    psum = ctx.enter_context(tc.tile_pool(name="psum", bufs=2, space="PSUM"))
    small = ctx.enter_context(tc.tile_pool(name="small", bufs=4))

    # ---- input DMAs. x (fp16) rides the Activation HWDGE ring:
    # ScalarE is idle until the first ACTIVATE so the trigger cost is
    # free there, and x does not queue behind the output stream on the
    # SP ring. W (tiny, fp32) keeps the SP ring.
    x_view = x_ap.rearrange("(t p) s -> p t s", p=128)
    x_sb = data.tile([128, 2, HW], PROJ_DT)
    nc.scalar.dma_start(
        out=x_sb[:, :, 0:512], in_=x_view[:, :, 0:512]
    )
    w_sb = const.tile([128, 2, C], F32)
    nc.sync.dma_start(out=w_sb, in_=w_ap.rearrange("(t p) c -> p t c", p=128))
    for lo, hi in X_DMA[1:]:
        nc.scalar.dma_start(out=x_sb[:, :, lo:hi], in_=x_view[:, :, lo:hi])

    # ---- PE warm-up: throwaway matmuls while x is loading. The PE
    # clock (HAM) only ramps after sustained activity; warming during
    # the input DMA makes the projection and the first attention
    # row-tiles run at full rate.
    warm_f32 = const.tile([128, BANK], F32)
    nc.vector.memset(warm_f32, 0.0)
    warm = const.tile([128, BANK], MM_DT)
    nc.vector.tensor_copy(out=warm, in_=warm_f32)
    wps = psum.tile([128, ACT_CHUNK], F32, tag="ps")
    for _ in range(4):
        nc.tensor.matmul(
            wps[:, 0:BANK], warm[:, 0:128], warm, start=True, stop=True
        )

    ident = const.tile([128, 128], F32)
    make_identity(nc, ident)

    # pull the exp table load off the first real activation; the dummy
    # accum_out read resets the ACT accumulator register so the warm-up
    # exp(0)=1 does not leak into the first row's sum
    tbl = small.tile([128, 2], F32, tag="tbl")
    nc.scalar.activation(
        out=tbl[:, 0:1], in_=warm_f32[:, 0:1],
        func=mybir.ActivationFunctionType.Exp, accum_out=tbl[:, 1:2],
    )

    # ---- transpose W on PE -> wt_sb[c_sub, c_tile, o] (contraction c on partitions)
    wt_sb = const.tile([128, 2, 2 * D], PROJ_DT)
    for t in range(2):          # output-channel tile (q half / k half)
        for ct in range(2):     # input-channel tile
            ps = psum.tile([128, ACT_CHUNK], F32, tag="ps")
            nc.tensor.transpose(
                ps[:, 0:128], w_sb[:, t, ct * 128:(ct + 1) * 128], ident
            )
            nc.vector.tensor_copy(
                out=wt_sb[:, ct, t * 128:(t + 1) * 128], in_=ps[:, 0:128]
            )
    # keep the PE clock ramping while the first x chunk lands (kept
    # short: these sit in PE program order ahead of the projections)
    wps2 = psum.tile([128, ACT_CHUNK], F32, tag="ps")
    for _ in range(2):
        nc.tensor.matmul(
            wps2[:, 0:BANK], warm[:, 0:128], warm, start=True, stop=True
        )

    q_sb = data.tile([128, HW], MM_DT)
    k_sb = data.tile([128, HW], MM_DT)

    def proj_cols(t, dst, lo, hi):
        """Project output-channel half t (0=q, 1=k) for columns [lo, hi)
        (hi-lo <= 2048) in <=512-wide bank pieces."""
        ps = psum.tile([128, ACT_CHUNK], F32, tag="ps")
        for j in range(0, hi - lo, BANK):
            n = min(BANK, hi - lo - j)
            sl = slice(lo + j, lo + j + n)
            psl = slice(j, j + n)
            for ct in range(2):
                nc.tensor.matmul(
                    ps[:, psl], wt_sb[:, ct, t * 128:(t + 1) * 128],
                    x_sb[:, ct, sl], start=(ct == 0), stop=(ct == 1),
                )
            nc.vector.tensor_copy(out=dst[:, sl], in_=ps[:, psl])

    outp = None
    # Query-row interleave: tile t of group g covers query rows
    # g*256 + 2p + t (p = partition). Per partition the two tiles are
    # CONSECUTIVE DRAM rows -> one contiguous 16 KB descriptor per
    # partition in the group DMA. 8 KB descriptors (row-per-partition)
    # cap the write drain at ~250 GB/s; 16 KB reach ~358 GB/s.
    out_view = out_ap.rearrange("(g p t) m -> g p t m", t=GRP, p=128)

    def tile_lhs(g, t):
        lo = g * GRP * 128 + t
        return q_sb[:, lo:lo + (GRP * 128 - GRP + 1):GRP]

    def sim_chunk(lhs, out_row, lo_col, n_col, accum):
        """n_col-wide slice of one attention row: matmuls + fused exp.

        Row sums come from the ACTIVATE's per-instruction accumulator
        (it does NOT persist across ACTIVATEs -- measured), costing a
        ~290ns ACTIVATION_READ_ACCUMULATOR on ScalarE per chunk. All
        DVE-side alternatives were MEASURED slower: tensor_reduce has
        no perf mode (2.13us/chunk, +11us end to end) and the
        tensor_scalar+accum_out variant, despite the IR model claiming
        4x_2p eligibility, runs at 1x on hardware (2.29us/chunk; the
        reduce tree takes one element per cycle) -- pushing DVE to 85%
        busy and +60us end to end. The accumulator read is genuinely
        the cheapest summer."""
        ps = psum.tile([128, ACT_CHUNK], F32, tag="ps")
        for jj in range(0, n_col, BANK):
            n = min(BANK, n_col - jj)
            sl = slice(lo_col + jj, lo_col + jj + n)
            nc.tensor.matmul(
                ps[:, jj:jj + n], lhs, k_sb[:, sl],
                start=True, stop=True,
            )
        sl = slice(lo_col, lo_col + n_col)
        nc.scalar.activation(
            out=out_row[:, sl],
            in_=ps[:, 0:n_col],
            func=mybir.ActivationFunctionType.Exp,
            scale=SCALE,
            accum_out=accum,
        )

    def normalize_tile(out_grp, g, t, rsum, split_dma):
        recip = small.tile([128, 1], F32, tag="recip")
        nc.vector.reciprocal(out=recip, in_=rsum)
        lo = g * GRP * 128 + t
        hi = lo + GRP * 128 - GRP + 1
        if split_dma == "half":
            # normalize and ship each half-row as soon as it is scaled
            # (0.5 MB transfers, 4 KB descriptors): last group only,
            # for the shortest possible tail. The two halves go to
            # different queues (SP + GpSimd SWDGE) so they drain in
            # parallel.
            for a in range(N_ACT):
                sl = slice(a * ACT_CHUNK, (a + 1) * ACT_CHUNK)
                nc.vector.tensor_scalar_mul(
                    out=out_grp[:, t, sl], in0=out_grp[:, t, sl],
                    scalar1=recip,
                )
                eng = nc.sync if a == 0 else nc.gpsimd
                eng.dma_start(
                    out=out_ap[lo:hi:GRP, sl],
                    in_=out_grp[:, t, sl],
                )
        else:
            nc.vector.tensor_scalar_mul(
                out=out_grp[:, t, :], in0=out_grp[:, t, :], scalar1=recip
            )
            if split_dma == "early":
                # The first group's tiles leave via GpSimd SWDGE: the
                # engine is otherwise idle, SWDGE uses separate queue
                # slots, and these 4 MB have ~100us of slack. Keeping
                # them out of the SP-ring FIFO means the output queue
                # never builds the ramp-time backlog that otherwise
                # takes ~13us to drain after the last ACTIVATE.
                nc.gpsimd.dma_start(
                    out=out_ap[lo:hi:GRP, :], in_=out_grp[:, t, :]
                )
            elif split_dma == "tile":
                # per-tile 1 MB HWDGE DMAs (8 KB descriptors)
                nc.sync.dma_start(
                    out=out_ap[lo:hi:GRP, :], in_=out_grp[:, t, :]
                )
            elif split_dma == "tile2q":
                # tail groups: alternate tiles between the SP ring and
                # the GpSimd SWDGE queue so the final ~8 MB drains on
                # two queues in parallel (one queue takes ~11us for
                # the last group alone, all of it after the last EXP)
                eng = nc.sync if t % 2 == 0 else nc.gpsimd
                eng.dma_start(
                    out=out_ap[lo:hi:GRP, :], in_=out_grp[:, t, :]
                )
            elif split_dma == "tile3q":
                # last group: three queues. Tile 0 rides the Activation
                # HWDGE ring -- idle since the x load -- with the
                # trigger emitted right here so ScalarE reaches it
                # during tile 1's EXPs (its normalize-mul wait already
                # satisfied; emitted any later it would head-of-line
                # block the remaining EXPs in the in-order queue).
                eng = (nc.scalar, nc.gpsimd, nc.sync)[t % 3]
                eng.dma_start(
                    out=out_ap[lo:hi:GRP, :], in_=out_grp[:, t, :]
                )

    def emit_group(g, split_dma=None, fine_tail=False, mid=None):
        out_grp = outp.tile([128, GRP, HW], F16, tag="out")
        for t in range(GRP):
            if t == 2 and mid is not None:
                # next group's q projection, emitted mid-group: by the
                # time group g+1's first sim matmul needs it, the PE
                # matmuls and DVE copies have long cleared -- issued
                # at the group boundary they cost a ~500ns ScalarE gap
                # per group (q-copy latency on the in-order DVE queue)
                mid()
            lhs = tile_lhs(g, t)
            fine = fine_tail and t == GRP - 1
            n_sum = 3 if fine else N_ACT
            sums = small.tile([128, n_sum], F32, tag="sums")
            sim_chunk(lhs, out_grp[:, t], 0, ACT_CHUNK, sums[:, 0:1])
            if fine:
                # split the very last chunk so the final exp->normalize->
                # DMA tail is as short as possible
                sim_chunk(lhs, out_grp[:, t], ACT_CHUNK, 1024,
                          sums[:, 1:2])
                sim_chunk(lhs, out_grp[:, t], ACT_CHUNK + 1024, 1024,
                          sums[:, 2:3])
            else:
                sim_chunk(lhs, out_grp[:, t], ACT_CHUNK, ACT_CHUNK,
                          sums[:, 1:2])
            rsum = small.tile([128, 1], F32, tag="rsum")
            nc.vector.tensor_reduce(
                out=rsum, in_=sums, axis=mybir.AxisListType.X,
                op=mybir.AluOpType.add,
            )
            # the very last tile ships as two pipelined 0.5 MB halves
            # so the final bytes leave ~1.5us sooner than one 1 MB DMA
            normalize_tile(out_grp, g, t, rsum,
                           "half" if fine else split_dma)
        if not split_dma:
            nc.sync.dma_start(out=out_view[g], in_=out_grp)

    def emit_early_groups():
        """Groups 0-1, reordered chunk-major: all four row-tiles' lower
        (cols 0:2048) chunks run first -- they only need the first half
        of x -- bridging ScalarE across the ~18 us it takes the upper
        half of x to arrive; the upper chunks and the normalizes follow.
        The very first row-tile runs 512/512/1024-wide so the first
        ACTIVATE fires as soon as the first 512 columns of x land."""
        og = outp.tile([128, GRP, HW], F16, tag="out", name="og0")
        sums = [small.tile([128, 4], F32, tag="sums", name=f"esums{i}")
                for i in range(4)]
        lhs = [tile_lhs(0, i) for i in range(4)]
        rows = [og[:, i] for i in range(4)]
        # column-major across the four tiles: each wave of chunks only
        # needs the slice of x/k available by the time ScalarE gets
        # there. The PE runs at ~0.81 GHz until t~20us (fixed DVFS
        # ramp), so the remaining projection work is fed in 1-2 piece
        # slivers BETWEEN waves -- one long projection train anywhere
        # in the first 20us starves the EXP stream for its full length
        # (measured 6.6us emitted after c0, 7.5us emitted before it).
        for i in range(4):
            sim_chunk(lhs[i], rows[i], 0, 512, sums[i][:, 0:1])
        proj_cols(1, k_sb, 512, 1024)
        for i in range(4):
            sim_chunk(lhs[i], rows[i], 512, 512, sums[i][:, 1:2])
        proj_cols(1, k_sb, 1024, 2048)
        proj_cols(1, k_sb, 2048, 3072)
        for i in range(4):
            sim_chunk(lhs[i], rows[i], 1024, 1024, sums[i][:, 2:3])
        proj_cols(1, k_sb, 3072, 4096)
        proj_cols(0, q_sb, 512, 1024)
        for i in range(4):
            sim_chunk(lhs[i], rows[i], 2048, 2048, sums[i][:, 3:4])
            rsum = small.tile([128, 1], F32, tag="rsum")
            nc.vector.tensor_reduce(
                out=rsum, in_=sums[i], axis=mybir.AxisListType.X,
                op=mybir.AluOpType.add,
            )
            normalize_tile(og, 0, i, rsum, "early")

    # ---- projection, interleaved with the attention groups so the
    # in-order PE reaches the first ACTIVATE as early as possible.
    proj_cols(1, k_sb, 0, 512)      # k cols 0:512 (first x chunk)
    proj_cols(0, q_sb, 0, 512)      # q rows 0:512 -> groups 0-1

    outp = ctx.enter_context(tc.tile_pool(name="outp", bufs=OUT_BUFS))
    emit_early_groups()
    # group g consumes q columns [g*512, (g+1)*512); each group's q
    # window is projected during the PREVIOUS group (mid=), a full
    # group ahead of its first consumer, so the group-boundary sim
    # matmul never waits on the q PSUM->SBUF copy.
    # Ship plan: the single SP-ring queue bursts at ~390 GB/s but a
    # whole 4 MB group bunches right at that group's normalize, so
    # group 5 rides the otherwise-idle GpSimd SWDGE queue and the two
    # tail groups alternate tiles across both queues -- the final ~8 MB
    # drains in parallel instead of serially after the last EXP.
    # group 1's q window was already projected inside the early phase
    ship = {1: "tile", 5: "early", 6: "tile2q", 7: "tile2q"}
    for g in range(1, N_GRP):
        mid = None
        if g < N_GRP - 1:
            def mid(gg=g):
                proj_cols(0, q_sb, (gg + 1) * 512, (gg + 2) * 512)
        emit_group(g, split_dma=ship.get(g),
                   fine_tail=(g == N_GRP - 1), mid=mid)


_built = None


def _get_nc():
    global _built
    if _built is None:
        nc = bacc.Bacc("TRN2", target_bir_lowering=False, debug=False)
        x = nc.dram_tensor("x", [C, HW], F16, kind="ExternalInput").ap()
        w = nc.dram_tensor("w", [2 * D, C], F32, kind="ExternalInput").ap()
        out = nc.dram_tensor("out", [HW, HW], F16, kind="ExternalOutput").ap()
        with tile.TileContext(nc) as tc:
            with ExitStack() as ctx:
                _emit(ctx, tc, out, x, w)
        nc.compile()
        _built = nc
    return _built


def kernel(x: np.ndarray, W: np.ndarray) -> np.ndarray:
    nc = _get_nc()
    # x ships as fp16: the kernel previously cast it on-chip before any
    # use, so this is numerically identical and halves the input DMA
    x = np.asarray(x, dtype=np.float32).astype(np.float16)
    W = np.ascontiguousarray(np.asarray(W, dtype=np.float32))
    in_maps = [
        {"x": np.ascontiguousarray(x[b].reshape(C, HW)), "w": W} for b in range(B)
    ]
    res = run_bass_kernel_spmd(nc, in_maps, core_ids=list(range(N_CORES)))
    out = np.stack(
        [res.results[b]["out"].astype(np.float32) for b in range(B)]
    )
    return out[:, None]



# revision 32
# speedup vs baseline: 1.0050x; 1.0050x over previous
"""Spatial self-attention scores kernel for Trainium2 (8 NeuronCores).

Computes, per batch b:
    qk = W @ x_b          # [256, 4096] = [256,256] @ [256,4096]
    q, k = qk[:128], qk[128:]
    sim = (q.T @ k) * 128**-0.5
    out_b = softmax(sim, axis=-1)        # [4096, 4096]
Output: [8, 1, 4096, 4096] float32.

Sharding: data-parallel over batch, one batch image per NeuronCore.

The kernel is ScalarE-bound: softmax's exp runs only on the scalar
engine (1 elem/cycle/lane @ 1.2 GHz => ~109 us body + ~27 us of
instruction/accumulator-read overhead for the 16.7M outputs per
core), so every other phase is arranged to hide under it:
  - x arrives as fp32 via three HWDGE DMAs on the Activation ring
    (ScalarE is idle pre-ramp, and this keeps x off the output ring)
    and is cast fp32->fp16 on DVE in 512-col pieces emitted at their
    point of use (up-front emission head-of-line blocks the in-order
    DVE queue on late x chunks). The ~12 us x transfer time is the
    startup floor.
  - fp16 projection matmuls -> q,k in SBUF as [d=128, s=4096] fp16,
    interleaved with the attention groups; PE warm-up matmuls keep
    the HAM clock ramping while x lands.
  - per 128-query row-tile: fp16 matmuls (K=128, N=512) into 4-bank
    PSUM tiles; one ScalarE ACTIVATE per 2048 columns computes
    exp(SCALE*sim) straight to fp16 with a fused row-sum (accum_out;
    measured cheaper than any DVE-side reduction); DVE combines the
    partial sums, takes the reciprocal, scales the row.
  - the first four row-tiles run column-major (each 512/1536/2048
    wave starts as soon as the x slice it needs has landed), so the
    in-order ACTIVATE queue never waits on the x DMA.
  - query rows interleave across output-group tiles (tile t of group
    g = rows g*512 + 4p + t): each partition then holds 4 consecutive
    DRAM rows = one contiguous 32 KB descriptor per partition per
    4 MB group DMA. Small descriptors otherwise cap the write drain
    at ~250 GB/s (vs ~340 measured with large ones).
  - output leaves as fp16 and is upcast to fp32 on the host (rel-err
    cost ~5e-4 against a 2e-2 budget): halves the HBM write traffic
    that roofline-bound the fp32 version at ~187 us/core; the last
    group ships per normalized half-row to shorten the tail.
"""

import numpy as np
from contextlib import ExitStack

import concourse.bass as bass
import concourse.tile as tile
from concourse import bacc, mybir
from concourse.bass_utils import run_bass_kernel_spmd
from concourse.masks import make_identity

B = 8
C = 256
HW = 4096
D = 128
SCALE = D ** -0.5
N_CORES = 8

BANK = 512             # PSUM bank width (fp32) = one matmul free-dim
ACT_CHUNK = 2048       # one ScalarE activation spans 4 banks
N_ACT = HW // ACT_CHUNK          # 2
GRP = 4                # row-tiles per output DMA (4 -> 4 MB fp16 transfers)
N_GRP = HW // (128 * GRP)        # 8
OUT_BUFS = 4

F32 = mybir.dt.float32
F16 = mybir.dt.float16
MM_DT = mybir.dt.float16
PROJ_DT = mybir.dt.float16

# x input DMA chunks (fp16, HWDGE on the Activation ring). x ships
# from the host already cast to fp16: the kernel used to cast
# fp32->fp16 on DVE before any use, so numerics are identical, but
# the input load halves to 2.1 MB (~6.5 us at the ~330 GB/s per-core
# read path -- splitting across two queues was measured useless, the
# queues just share the same path at half rate each). The first 512
# columns land alone so the first attention wave can start.
X_DMA = ((0, 512), (512, 2048), (2048, 4096))


def _emit(ctx: ExitStack, tc: tile.TileContext, out_ap, x_ap, w_ap):
    nc = tc.nc

    const = ctx.enter_context(tc.tile_pool(name="const", bufs=1))
    data = ctx.enter_context(tc.tile_pool(name="data", bufs=1))
    psum = ctx.enter_context(tc.tile_pool(name="psum", bufs=2, space="PSUM"))
    small = ctx.enter_context(tc.tile_pool(name="small", bufs=4))

    # ---- input DMAs. x (fp16) rides the Activation HWDGE ring:
    # ScalarE is idle until the first ACTIVATE so the trigger cost is
    # free there, and x does not queue behind the output stream on the
    # SP ring. W (tiny, fp32) keeps the SP ring.
    x_view = x_ap.rearrange("(t p) s -> p t s", p=128)
    x_sb = data.tile([128, 2, HW], PROJ_DT)
    nc.scalar.dma_start(
        out=x_sb[:, :, 0:512], in_=x_view[:, :, 0:512]
    )
    w_sb = const.tile([128, 2, C], F32)
    nc.sync.dma_start(out=w_sb, in_=w_ap.rearrange("(t p) c -> p t c", p=128))
    for lo, hi in X_DMA[1:]:
        nc.scalar.dma_start(out=x_sb[:, :, lo:hi], in_=x_view[:, :, lo:hi])

    # ---- PE warm-up: throwaway matmuls while x is loading. The PE
    # clock (HAM) only ramps after sustained activity; warming during
    # the input DMA makes the projection and the first attention
    # row-tiles run at full rate.
    warm_f32 = const.tile([128, BANK], F32)
    nc.vector.memset(warm_f32, 0.0)
    warm = const.tile([128, BANK], MM_DT)
    nc.vector.tensor_copy(out=warm, in_=warm_f32)
    wps = psum.tile([128, ACT_CHUNK], F32, tag="ps")
    for _ in range(4):
        nc.tensor.matmul(
            wps[:, 0:BANK], warm[:, 0:128], warm, start=True, stop=True
        )

    ident = const.tile([128, 128], F32)
    make_identity(nc, ident)

    # pull the exp table load off the first real activation; the dummy
    # accum_out read resets the ACT accumulator register so the warm-up
    # exp(0)=1 does not leak into the first row's sum
    tbl = small.tile([128, 2], F32, tag="tbl")
    nc.scalar.activation(
        out=tbl[:, 0:1], in_=warm_f32[:, 0:1],
        func=mybir.ActivationFunctionType.Exp, accum_out=tbl[:, 1:2],
    )

    # ---- transpose W on PE -> wt_sb[c_sub, c_tile, o] (contraction c on partitions)
    wt_sb = const.tile([128, 2, 2 * D], PROJ_DT)
    for t in range(2):          # output-channel tile (q half / k half)
        for ct in range(2):     # input-channel tile
            ps = psum.tile([128, ACT_CHUNK], F32, tag="ps")
            nc.tensor.transpose(
                ps[:, 0:128], w_sb[:, t, ct * 128:(ct + 1) * 128], ident
            )
            nc.vector.tensor_copy(
                out=wt_sb[:, ct, t * 128:(t + 1) * 128], in_=ps[:, 0:128]
            )
    # keep the PE clock ramping while the first x chunk lands (kept
    # short: these sit in PE program order ahead of the projections)
    wps2 = psum.tile([128, ACT_CHUNK], F32, tag="ps")
    for _ in range(2):
        nc.tensor.matmul(
            wps2[:, 0:BANK], warm[:, 0:128], warm, start=True, stop=True
        )

    q_sb = data.tile([128, HW], MM_DT)
    k_sb = data.tile([128, HW], MM_DT)

    def proj_cols(t, dst, lo, hi):
        """Project output-channel half t (0=q, 1=k) for columns [lo, hi)
        (hi-lo <= 2048) in <=512-wide bank pieces."""
        ps = psum.tile([128, ACT_CHUNK], F32, tag="ps")
        for j in range(0, hi - lo, BANK):
            n = min(BANK, hi - lo - j)
            sl = slice(lo + j, lo + j + n)
            psl = slice(j, j + n)
            for ct in range(2):
                nc.tensor.matmul(
                    ps[:, psl], wt_sb[:, ct, t * 128:(t + 1) * 128],
                    x_sb[:, ct, sl], start=(ct == 0), stop=(ct == 1),
                )
            nc.vector.tensor_copy(out=dst[:, sl], in_=ps[:, psl])

    outp = None
    # Query-row interleave: tile t of group g covers query rows
    # g*256 + 2p + t (p = partition). Per partition the two tiles are
    # CONSECUTIVE DRAM rows -> one contiguous 16 KB descriptor per
    # partition in the group DMA. 8 KB descriptors (row-per-partition)
    # cap the write drain at ~250 GB/s; 16 KB reach ~358 GB/s.
    out_view = out_ap.rearrange("(g p t) m -> g p t m", t=GRP, p=128)

    def tile_lhs(g, t):
        lo = g * GRP * 128 + t
        return q_sb[:, lo:lo + (GRP * 128 - GRP + 1):GRP]

    def sim_chunk(lhs, out_row, lo_col, n_col, accum):
        """n_col-wide slice of one attention row: matmuls + fused exp.

        Row sums come from the ACTIVATE's per-instruction accumulator
        (it does NOT persist across ACTIVATEs -- measured), costing a
        ~290ns ACTIVATION_READ_ACCUMULATOR on ScalarE per chunk. All
        DVE-side alternatives were MEASURED slower: tensor_reduce has
        no perf mode (2.13us/chunk, +11us end to end) and the
        tensor_scalar+accum_out variant, despite the IR model claiming
        4x_2p eligibility, runs at 1x on hardware (2.29us/chunk; the
        reduce tree takes one element per cycle) -- pushing DVE to 85%
        busy and +60us end to end. The accumulator read is genuinely
        the cheapest summer."""
        ps = psum.tile([128, ACT_CHUNK], F32, tag="ps")
        for jj in range(0, n_col, BANK):
            n = min(BANK, n_col - jj)
            sl = slice(lo_col + jj, lo_col + jj + n)
            nc.tensor.matmul(
                ps[:, jj:jj + n], lhs, k_sb[:, sl],
                start=True, stop=True,
            )
        sl = slice(lo_col, lo_col + n_col)
        nc.scalar.activation(
            out=out_row[:, sl],
            in_=ps[:, 0:n_col],
            func=mybir.ActivationFunctionType.Exp,
            scale=SCALE,
            accum_out=accum,
        )

    def normalize_tile(out_grp, g, t, rsum, split_dma):
        recip = small.tile([128, 1], F32, tag="recip")
        nc.vector.reciprocal(out=recip, in_=rsum)
        lo = g * GRP * 128 + t
        hi = lo + GRP * 128 - GRP + 1
        if split_dma == "half":
            # normalize and ship each half-row as soon as it is scaled
            # (0.5 MB transfers, 4 KB descriptors): last group only,
            # for the shortest possible tail. The two halves go to
            # different queues (SP + GpSimd SWDGE) so they drain in
            # parallel.
            for a in range(N_ACT):
                sl = slice(a * ACT_CHUNK, (a + 1) * ACT_CHUNK)
                nc.vector.tensor_scalar_mul(
                    out=out_grp[:, t, sl], in0=out_grp[:, t, sl],
                    scalar1=recip,
                )
                eng = nc.sync if a == 0 else nc.gpsimd
                eng.dma_start(
                    out=out_ap[lo:hi:GRP, sl],
                    in_=out_grp[:, t, sl],
                )
        else:
            nc.vector.tensor_scalar_mul(
                out=out_grp[:, t, :], in0=out_grp[:, t, :], scalar1=recip
            )
            if split_dma == "early":
                # The first group's tiles leave via GpSimd SWDGE: the
                # engine is otherwise idle, SWDGE uses separate queue
                # slots, and these 4 MB have ~100us of slack. Keeping
                # them out of the SP-ring FIFO means the output queue
                # never builds the ramp-time backlog that otherwise
                # takes ~13us to drain after the last ACTIVATE.
                nc.gpsimd.dma_start(
                    out=out_ap[lo:hi:GRP, :], in_=out_grp[:, t, :]
                )
            elif split_dma == "tile":
                # per-tile 1 MB HWDGE DMAs (8 KB descriptors)
                nc.sync.dma_start(
                    out=out_ap[lo:hi:GRP, :], in_=out_grp[:, t, :]
                )
            elif split_dma == "tile2q":
                # tail groups: alternate tiles between the SP ring and
                # the GpSimd SWDGE queue so the final ~8 MB drains on
                # two queues in parallel (one queue takes ~11us for
                # the last group alone, all of it after the last EXP)
                eng = nc.sync if t % 2 == 0 else nc.gpsimd
                eng.dma_start(
                    out=out_ap[lo:hi:GRP, :], in_=out_grp[:, t, :]
                )
            elif split_dma == "tile3q":
                # last group: three queues. Tile 0 rides the Activation
                # HWDGE ring -- idle since the x load -- with the
                # trigger emitted right here so ScalarE reaches it
                # during tile 1's EXPs (its normalize-mul wait already
                # satisfied; emitted any later it would head-of-line
                # block the remaining EXPs in the in-order queue).
                eng = (nc.scalar, nc.gpsimd, nc.sync)[t % 3]
                eng.dma_start(
                    out=out_ap[lo:hi:GRP, :], in_=out_grp[:, t, :]
                )

    def emit_group(g, split_dma=None, fine_tail=False, mid=None):
        out_grp = outp.tile([128, GRP, HW], F16, tag="out")
        for t in range(GRP):
            if t == 2 and mid is not None:
                # next group's q projection, emitted mid-group: by the
                # time group g+1's first sim matmul needs it, the PE
                # matmuls and DVE copies have long cleared -- issued
                # at the group boundary they cost a ~500ns ScalarE gap
                # per group (q-copy latency on the in-order DVE queue)
                mid()
            lhs = tile_lhs(g, t)
            fine = fine_tail and t == GRP - 1
            n_sum = 3 if fine else N_ACT
            sums = small.tile([128, n_sum], F32, tag="sums")
            sim_chunk(lhs, out_grp[:, t], 0, ACT_CHUNK, sums[:, 0:1])
            if fine:
                # split the very last chunk so the final exp->normalize->
                # DMA tail is as short as possible
                sim_chunk(lhs, out_grp[:, t], ACT_CHUNK, 1024,
                          sums[:, 1:2])
                sim_chunk(lhs, out_grp[:, t], ACT_CHUNK + 1024, 1024,
                          sums[:, 2:3])
            else:
                sim_chunk(lhs, out_grp[:, t], ACT_CHUNK, ACT_CHUNK,
                          sums[:, 1:2])
            rsum = small.tile([128, 1], F32, tag="rsum")
            nc.vector.tensor_reduce(
                out=rsum, in_=sums, axis=mybir.AxisListType.X,
                op=mybir.AluOpType.add,
            )
            # the very last tile ships as two pipelined 0.5 MB halves
            # so the final bytes leave ~1.5us sooner than one 1 MB DMA
            normalize_tile(out_grp, g, t, rsum,
                           "half" if fine else split_dma)
        if not split_dma:
            nc.sync.dma_start(out=out_view[g], in_=out_grp)

    def emit_early_groups():
        """Groups 0-1, reordered chunk-major: all four row-tiles' lower
        (cols 0:2048) chunks run first -- they only need the first half
        of x -- bridging ScalarE across the ~18 us it takes the upper
        half of x to arrive; the upper chunks and the normalizes follow.
        The very first row-tile runs 512/512/1024-wide so the first
        ACTIVATE fires as soon as the first 512 columns of x land."""
        og = outp.tile([128, GRP, HW], F16, tag="out", name="og0")
        sums = [small.tile([128, 3], F32, tag="sums", name=f"esums{i}")
                for i in range(4)]
        lhs = [tile_lhs(0, i) for i in range(4)]
        rows = [og[:, i] for i in range(4)]
        # column-major across the four tiles, 1024/1024/2048 waves.
        # The PE runs at ~0.81 GHz until t~20us (fixed DVFS ramp), so
        # the early phase is PE-supply-bound: the 1024-wide c0 wave
        # (~8.5us of EXP) starts as soon as k/q cols 0:1024 project
        # and bridges most of the low-clock window while the k/q
        # projection train runs underneath it. (A 512-wide c0 with the
        # train emitted after it left a measured 6.6us c1 gap; the
        # train emitted before c0 pushed the first EXP 7.5us later;
        # finer 512-wide interleaving was also worse -- the extra
        # ACTIVATE overhead and wave jitter cost ~3us.)
        proj_cols(1, k_sb, 512, 1024)
        for i in range(4):
            sim_chunk(lhs[i], rows[i], 0, 1024, sums[i][:, 0:1])
        proj_cols(1, k_sb, 1024, 2048)
        proj_cols(1, k_sb, 2048, 3072)
        proj_cols(1, k_sb, 3072, 4096)
        proj_cols(0, q_sb, 512, 1024)
        for i in range(4):
            sim_chunk(lhs[i], rows[i], 1024, 1024, sums[i][:, 1:2])
        for i in range(4):
            sim_chunk(lhs[i], rows[i], 2048, 2048, sums[i][:, 2:3])
            rsum = small.tile([128, 1], F32, tag="rsum")
            nc.vector.tensor_reduce(
                out=rsum, in_=sums[i], axis=mybir.AxisListType.X,
                op=mybir.AluOpType.add,
            )
            normalize_tile(og, 0, i, rsum, "early")

    # ---- projection, interleaved with the attention groups so the
    # in-order PE reaches the first ACTIVATE as early as possible.
    proj_cols(1, k_sb, 0, 512)      # k cols 0:512 (first x chunk)
    proj_cols(0, q_sb, 0, 512)      # q rows 0:512 -> groups 0-1

    outp = ctx.enter_context(tc.tile_pool(name="outp", bufs=OUT_BUFS))
    emit_early_groups()
    # group g consumes q columns [g*512, (g+1)*512); each group's q
    # window is projected during the PREVIOUS group (mid=), a full
    # group ahead of its first consumer, so the group-boundary sim
    # matmul never waits on the q PSUM->SBUF copy.
    # Ship plan: the single SP-ring queue bursts at ~390 GB/s but a
    # whole 4 MB group bunches right at that group's normalize, so
    # group 5 rides the otherwise-idle GpSimd SWDGE queue and the two
    # tail groups alternate tiles across both queues -- the final ~8 MB
    # drains in parallel instead of serially after the last EXP.
    # group 1's q window was already projected inside the early phase
    ship = {1: "tile", 5: "early", 6: "tile2q", 7: "tile2q"}
    for g in range(1, N_GRP):
        mid = None
        if g < N_GRP - 1:
            def mid(gg=g):
                proj_cols(0, q_sb, (gg + 1) * 512, (gg + 2) * 512)
        emit_group(g, split_dma=ship.get(g),
                   fine_tail=(g == N_GRP - 1), mid=mid)


_built = None


def _get_nc():
    global _built
    if _built is None:
        nc = bacc.Bacc("TRN2", target_bir_lowering=False, debug=False)
        x = nc.dram_tensor("x", [C, HW], F16, kind="ExternalInput").ap()
        w = nc.dram_tensor("w", [2 * D, C], F32, kind="ExternalInput").ap()
        out = nc.dram_tensor("out", [HW, HW], F16, kind="ExternalOutput").ap()
        with tile.TileContext(nc) as tc:
            with ExitStack() as ctx:
                _emit(ctx, tc, out, x, w)
        nc.compile()
        _built = nc
    return _built


def kernel(x: np.ndarray, W: np.ndarray) -> np.ndarray:
    nc = _get_nc()
    # x ships as fp16: the kernel previously cast it on-chip before any
    # use, so this is numerically identical and halves the input DMA
    x = np.asarray(x, dtype=np.float32).astype(np.float16)
    W = np.ascontiguousarray(np.asarray(W, dtype=np.float32))
    in_maps = [
        {"x": np.ascontiguousarray(x[b].reshape(C, HW)), "w": W} for b in range(B)
    ]
    res = run_bass_kernel_spmd(nc, in_maps, core_ids=list(range(N_CORES)))
    out = np.stack(
        [res.results[b]["out"].astype(np.float32) for b in range(B)]
    )
    return out[:, None]



# revision 34
# speedup vs baseline: 1.0077x; 1.0027x over previous
"""Spatial self-attention scores kernel for Trainium2 (8 NeuronCores).

Computes, per batch b:
    qk = W @ x_b          # [256, 4096] = [256,256] @ [256,4096]
    q, k = qk[:128], qk[128:]
    sim = (q.T @ k) * 128**-0.5
    out_b = softmax(sim, axis=-1)        # [4096, 4096]
Output: [8, 1, 4096, 4096] float32.

Sharding: data-parallel over batch, one batch image per NeuronCore.

The kernel is ScalarE-bound: softmax's exp runs only on the scalar
engine (1 elem/cycle/lane @ 1.2 GHz => ~109 us body + ~27 us of
instruction/accumulator-read overhead for the 16.7M outputs per
core), so every other phase is arranged to hide under it:
  - x arrives as fp32 via three HWDGE DMAs on the Activation ring
    (ScalarE is idle pre-ramp, and this keeps x off the output ring)
    and is cast fp32->fp16 on DVE in 512-col pieces emitted at their
    point of use (up-front emission head-of-line blocks the in-order
    DVE queue on late x chunks). The ~12 us x transfer time is the
    startup floor.
  - fp16 projection matmuls -> q,k in SBUF as [d=128, s=4096] fp16,
    interleaved with the attention groups; PE warm-up matmuls keep
    the HAM clock ramping while x lands.
  - per 128-query row-tile: fp16 matmuls (K=128, N=512) into 4-bank
    PSUM tiles; one ScalarE ACTIVATE per 2048 columns computes
    exp(SCALE*sim) straight to fp16 with a fused row-sum (accum_out;
    measured cheaper than any DVE-side reduction); DVE combines the
    partial sums, takes the reciprocal, scales the row.
  - the first four row-tiles run column-major (each 512/1536/2048
    wave starts as soon as the x slice it needs has landed), so the
    in-order ACTIVATE queue never waits on the x DMA.
  - query rows interleave across output-group tiles (tile t of group
    g = rows g*512 + 4p + t): each partition then holds 4 consecutive
    DRAM rows = one contiguous 32 KB descriptor per partition per
    4 MB group DMA. Small descriptors otherwise cap the write drain
    at ~250 GB/s (vs ~340 measured with large ones).
  - output leaves as fp16 and is upcast to fp32 on the host (rel-err
    cost ~5e-4 against a 2e-2 budget): halves the HBM write traffic
    that roofline-bound the fp32 version at ~187 us/core; the last
    group ships per normalized half-row to shorten the tail.
"""

import numpy as np
from contextlib import ExitStack

import concourse.bass as bass
import concourse.tile as tile
from concourse import bacc, mybir
from concourse.bass_utils import run_bass_kernel_spmd
from concourse.masks import make_identity

B = 8
C = 256
HW = 4096
D = 128
SCALE = D ** -0.5
N_CORES = 8

BANK = 512             # PSUM bank width (fp32) = one matmul free-dim
ACT_CHUNK = 2048       # one ScalarE activation spans 4 banks
N_ACT = HW // ACT_CHUNK          # 2
GRP = 4                # row-tiles per output DMA (4 -> 4 MB fp16 transfers)
N_GRP = HW // (128 * GRP)        # 8
OUT_BUFS = 3

F32 = mybir.dt.float32
F16 = mybir.dt.float16
MM_DT = mybir.dt.float16
PROJ_DT = mybir.dt.float16

# x input DMA chunks (fp16, HWDGE on the Activation ring). x ships
# from the host already cast to fp16: the kernel used to cast
# fp32->fp16 on DVE before any use, so numerics are identical, but
# the input load halves to 2.1 MB (~6.5 us at the ~330 GB/s per-core
# read path -- splitting across two queues was measured useless, the
# queues just share the same path at half rate each). The first 512
# columns land alone so the first attention wave can start.
X_DMA = ((0, 512), (512, 2048), (2048, 4096))


def _emit(ctx: ExitStack, tc: tile.TileContext, out_ap, x_ap, w_ap):
    nc = tc.nc

    const = ctx.enter_context(tc.tile_pool(name="const", bufs=1))
    data = ctx.enter_context(tc.tile_pool(name="data", bufs=1))
    psum = ctx.enter_context(tc.tile_pool(name="psum", bufs=2, space="PSUM"))
    small = ctx.enter_context(tc.tile_pool(name="small", bufs=4))

    # ---- input DMAs. x (fp16) rides the Activation HWDGE ring:
    # ScalarE is idle until the first ACTIVATE so the trigger cost is
    # free there, and x does not queue behind the output stream on the
    # SP ring. W (tiny, fp32) keeps the SP ring.
    x_view = x_ap.rearrange("(t p) s -> p t s", p=128)
    x_sb = data.tile([128, 2, HW], PROJ_DT)
    nc.scalar.dma_start(
        out=x_sb[:, :, 0:512], in_=x_view[:, :, 0:512]
    )
    w_sb = const.tile([128, 2, C], F32)
    nc.sync.dma_start(out=w_sb, in_=w_ap.rearrange("(t p) c -> p t c", p=128))
    for lo, hi in X_DMA[1:]:
        nc.scalar.dma_start(out=x_sb[:, :, lo:hi], in_=x_view[:, :, lo:hi])

    # ---- PE warm-up: throwaway matmuls while x is loading. The PE
    # clock (HAM) only ramps after sustained activity; warming during
    # the input DMA makes the projection and the first attention
    # row-tiles run at full rate.
    warm_f32 = const.tile([128, BANK], F32)
    nc.vector.memset(warm_f32, 0.0)
    warm = const.tile([128, BANK], MM_DT)
    nc.vector.tensor_copy(out=warm, in_=warm_f32)
    wps = psum.tile([128, ACT_CHUNK], F32, tag="ps")
    for _ in range(4):
        nc.tensor.matmul(
            wps[:, 0:BANK], warm[:, 0:128], warm, start=True, stop=True
        )

    ident = const.tile([128, 128], F32)
    make_identity(nc, ident)

    # pull the exp table load off the first real activation; the dummy
    # accum_out read resets the ACT accumulator register so the warm-up
    # exp(0)=1 does not leak into the first row's sum
    tbl = small.tile([128, 2], F32, tag="tbl")
    nc.scalar.activation(
        out=tbl[:, 0:1], in_=warm_f32[:, 0:1],
        func=mybir.ActivationFunctionType.Exp, accum_out=tbl[:, 1:2],
    )

    # ---- transpose W on PE -> wt_sb[c_sub, c_tile, o] (contraction c on partitions)
    wt_sb = const.tile([128, 2, 2 * D], PROJ_DT)
    for t in range(2):          # output-channel tile (q half / k half)
        for ct in range(2):     # input-channel tile
            ps = psum.tile([128, ACT_CHUNK], F32, tag="ps")
            nc.tensor.transpose(
                ps[:, 0:128], w_sb[:, t, ct * 128:(ct + 1) * 128], ident
            )
            nc.vector.tensor_copy(
                out=wt_sb[:, ct, t * 128:(t + 1) * 128], in_=ps[:, 0:128]
            )
    # keep the PE clock ramping while the first x chunk lands (kept
    # short: these sit in PE program order ahead of the projections)
    wps2 = psum.tile([128, ACT_CHUNK], F32, tag="ps")
    for _ in range(2):
        nc.tensor.matmul(
            wps2[:, 0:BANK], warm[:, 0:128], warm, start=True, stop=True
        )

    q_sb = data.tile([128, HW], MM_DT)
    k_sb = data.tile([128, HW], MM_DT)

    def proj_cols(t, dst, lo, hi):
        """Project output-channel half t (0=q, 1=k) for columns [lo, hi)
        (hi-lo <= 2048) in <=512-wide bank pieces."""
        ps = psum.tile([128, ACT_CHUNK], F32, tag="ps")
        for j in range(0, hi - lo, BANK):
            n = min(BANK, hi - lo - j)
            sl = slice(lo + j, lo + j + n)
            psl = slice(j, j + n)
            for ct in range(2):
                nc.tensor.matmul(
                    ps[:, psl], wt_sb[:, ct, t * 128:(t + 1) * 128],
                    x_sb[:, ct, sl], start=(ct == 0), stop=(ct == 1),
                )
            nc.vector.tensor_copy(out=dst[:, sl], in_=ps[:, psl])

    outp = None
    # Query-row interleave: tile t of group g covers query rows
    # g*256 + 2p + t (p = partition). Per partition the two tiles are
    # CONSECUTIVE DRAM rows -> one contiguous 16 KB descriptor per
    # partition in the group DMA. 8 KB descriptors (row-per-partition)
    # cap the write drain at ~250 GB/s; 16 KB reach ~358 GB/s.
    out_view = out_ap.rearrange("(g p t) m -> g p t m", t=GRP, p=128)

    def tile_lhs(g, t):
        lo = g * GRP * 128 + t
        return q_sb[:, lo:lo + (GRP * 128 - GRP + 1):GRP]

    def sim_chunk(lhs, out_row, lo_col, n_col, accum):
        """n_col-wide slice of one attention row: matmuls + fused exp.

        Row sums come from the ACTIVATE's per-instruction accumulator
        (it does NOT persist across ACTIVATEs -- measured), costing a
        ~290ns ACTIVATION_READ_ACCUMULATOR on ScalarE per chunk. All
        DVE-side alternatives were MEASURED slower: tensor_reduce has
        no perf mode (2.13us/chunk, +11us end to end) and the
        tensor_scalar+accum_out variant, despite the IR model claiming
        4x_2p eligibility, runs at 1x on hardware (2.29us/chunk; the
        reduce tree takes one element per cycle) -- pushing DVE to 85%
        busy and +60us end to end. The accumulator read is genuinely
        the cheapest summer."""
        ps = psum.tile([128, ACT_CHUNK], F32, tag="ps")
        for jj in range(0, n_col, BANK):
            n = min(BANK, n_col - jj)
            sl = slice(lo_col + jj, lo_col + jj + n)
            nc.tensor.matmul(
                ps[:, jj:jj + n], lhs, k_sb[:, sl],
                start=True, stop=True,
            )
        sl = slice(lo_col, lo_col + n_col)
        nc.scalar.activation(
            out=out_row[:, sl],
            in_=ps[:, 0:n_col],
            func=mybir.ActivationFunctionType.Exp,
            scale=SCALE,
            accum_out=accum,
        )

    def normalize_tile(out_grp, g, t, rsum, split_dma):
        recip = small.tile([128, 1], F32, tag="recip")
        nc.vector.reciprocal(out=recip, in_=rsum)
        lo = g * GRP * 128 + t
        hi = lo + GRP * 128 - GRP + 1
        if split_dma == "half":
            # normalize and ship each half-row as soon as it is scaled
            # (0.5 MB transfers, 4 KB descriptors): last group only,
            # for the shortest possible tail. The two halves go to
            # different queues (SP + GpSimd SWDGE) so they drain in
            # parallel.
            for a in range(N_ACT):
                sl = slice(a * ACT_CHUNK, (a + 1) * ACT_CHUNK)
                nc.vector.tensor_scalar_mul(
                    out=out_grp[:, t, sl], in0=out_grp[:, t, sl],
                    scalar1=recip,
                )
                eng = nc.sync if a == 0 else nc.gpsimd
                eng.dma_start(
                    out=out_ap[lo:hi:GRP, sl],
                    in_=out_grp[:, t, sl],
                )
        else:
            nc.vector.tensor_scalar_mul(
                out=out_grp[:, t, :], in0=out_grp[:, t, :], scalar1=recip
            )
            if split_dma == "early":
                # The first group's tiles leave via GpSimd SWDGE: the
                # engine is otherwise idle, SWDGE uses separate queue
                # slots, and these 4 MB have ~100us of slack. Keeping
                # them out of the SP-ring FIFO means the output queue
                # never builds the ramp-time backlog that otherwise
                # takes ~13us to drain after the last ACTIVATE.
                nc.gpsimd.dma_start(
                    out=out_ap[lo:hi:GRP, :], in_=out_grp[:, t, :]
                )
            elif split_dma == "tile":
                # per-tile 1 MB HWDGE DMAs (8 KB descriptors)
                nc.sync.dma_start(
                    out=out_ap[lo:hi:GRP, :], in_=out_grp[:, t, :]
                )
            elif split_dma == "tile2q":
                # tail groups: alternate tiles between the SP ring and
                # the GpSimd SWDGE queue so the final ~8 MB drains on
                # two queues in parallel (one queue takes ~11us for
                # the last group alone, all of it after the last EXP)
                eng = nc.sync if t % 2 == 0 else nc.gpsimd
                eng.dma_start(
                    out=out_ap[lo:hi:GRP, :], in_=out_grp[:, t, :]
                )
            elif split_dma == "tile3q":
                # last group: three queues. Tile 0 rides the Activation
                # HWDGE ring -- idle since the x load -- with the
                # trigger emitted right here so ScalarE reaches it
                # during tile 1's EXPs (its normalize-mul wait already
                # satisfied; emitted any later it would head-of-line
                # block the remaining EXPs in the in-order queue).
                eng = (nc.scalar, nc.gpsimd, nc.sync)[t % 3]
                eng.dma_start(
                    out=out_ap[lo:hi:GRP, :], in_=out_grp[:, t, :]
                )

    def emit_group(g, split_dma=None, fine_tail=False, mid=None):
        out_grp = outp.tile([128, GRP, HW], F16, tag="out")
        for t in range(GRP):
            if t == 2 and mid is not None:
                # next group's q projection, emitted mid-group: by the
                # time group g+1's first sim matmul needs it, the PE
                # matmuls and DVE copies have long cleared -- issued
                # at the group boundary they cost a ~500ns ScalarE gap
                # per group (q-copy latency on the in-order DVE queue)
                mid()
            lhs = tile_lhs(g, t)
            fine = fine_tail and t == GRP - 1
            n_sum = 3 if fine else N_ACT
            sums = small.tile([128, n_sum], F32, tag="sums")
            sim_chunk(lhs, out_grp[:, t], 0, ACT_CHUNK, sums[:, 0:1])
            if fine:
                # split the very last chunk so the final exp->normalize->
                # DMA tail is as short as possible
                sim_chunk(lhs, out_grp[:, t], ACT_CHUNK, 1024,
                          sums[:, 1:2])
                sim_chunk(lhs, out_grp[:, t], ACT_CHUNK + 1024, 1024,
                          sums[:, 2:3])
            else:
                sim_chunk(lhs, out_grp[:, t], ACT_CHUNK, ACT_CHUNK,
                          sums[:, 1:2])
            rsum = small.tile([128, 1], F32, tag="rsum")
            nc.vector.tensor_reduce(
                out=rsum, in_=sums, axis=mybir.AxisListType.X,
                op=mybir.AluOpType.add,
            )
            # the very last tile ships as two pipelined 0.5 MB halves
            # so the final bytes leave ~1.5us sooner than one 1 MB DMA
            normalize_tile(out_grp, g, t, rsum,
                           "half" if fine else split_dma)
        if not split_dma:
            nc.sync.dma_start(out=out_view[g], in_=out_grp)

    def emit_early_groups():
        """Groups 0-1, reordered chunk-major: all four row-tiles' lower
        (cols 0:2048) chunks run first -- they only need the first half
        of x -- bridging ScalarE across the ~18 us it takes the upper
        half of x to arrive; the upper chunks and the normalizes follow.
        The very first row-tile runs 512/512/1024-wide so the first
        ACTIVATE fires as soon as the first 512 columns of x land."""
        og = outp.tile([128, GRP, HW], F16, tag="out", name="og0")
        sums = [small.tile([128, 3], F32, tag="sums", name=f"esums{i}")
                for i in range(4)]
        lhs = [tile_lhs(0, i) for i in range(4)]
        rows = [og[:, i] for i in range(4)]
        # column-major across the four tiles, 1024/1024/2048 waves.
        # The PE runs at ~0.81 GHz until t~20us (fixed DVFS ramp), so
        # the early phase is PE-supply-bound: the 1024-wide c0 wave
        # (~8.5us of EXP) starts as soon as k/q cols 0:1024 project
        # and bridges most of the low-clock window while the k/q
        # projection train runs underneath it. (A 512-wide c0 with the
        # train emitted after it left a measured 6.6us c1 gap; the
        # train emitted before c0 pushed the first EXP 7.5us later;
        # finer 512-wide interleaving was also worse -- the extra
        # ACTIVATE overhead and wave jitter cost ~3us.)
        proj_cols(1, k_sb, 512, 1024)
        for i in range(4):
            sim_chunk(lhs[i], rows[i], 0, 1024, sums[i][:, 0:1])
        proj_cols(1, k_sb, 1024, 2048)
        proj_cols(1, k_sb, 2048, 3072)
        proj_cols(1, k_sb, 3072, 4096)
        proj_cols(0, q_sb, 512, 1024)
        for i in range(4):
            sim_chunk(lhs[i], rows[i], 1024, 1024, sums[i][:, 1:2])
        for i in range(4):
            sim_chunk(lhs[i], rows[i], 2048, 2048, sums[i][:, 2:3])
            rsum = small.tile([128, 1], F32, tag="rsum")
            nc.vector.tensor_reduce(
                out=rsum, in_=sums[i], axis=mybir.AxisListType.X,
                op=mybir.AluOpType.add,
            )
            normalize_tile(og, 0, i, rsum, "early")

    # ---- projection, interleaved with the attention groups so the
    # in-order PE reaches the first ACTIVATE as early as possible.
    proj_cols(1, k_sb, 0, 512)      # k cols 0:512 (first x chunk)
    proj_cols(0, q_sb, 0, 512)      # q rows 0:512 -> groups 0-1

    outp = ctx.enter_context(tc.tile_pool(name="outp", bufs=OUT_BUFS))
    emit_early_groups()
    # group g consumes q columns [g*512, (g+1)*512); each group's q
    # window is projected during the PREVIOUS group (mid=), a full
    # group ahead of its first consumer, so the group-boundary sim
    # matmul never waits on the q PSUM->SBUF copy.
    # Ship plan: the single SP-ring queue bursts at ~390 GB/s but a
    # whole 4 MB group bunches right at that group's normalize, so
    # group 5 rides the otherwise-idle GpSimd SWDGE queue and the two
    # tail groups alternate tiles across both queues -- the final ~8 MB
    # drains in parallel instead of serially after the last EXP.
    # group 1's q window was already projected inside the early phase
    ship = {1: "tile", 5: "early", 6: "tile2q"}
    for g in range(1, N_GRP - 1):
        def mid(gg=g):
            proj_cols(0, q_sb, (gg + 1) * 512, (gg + 2) * 512)
        emit_group(g, split_dma=ship.get(g), mid=mid)

    def emit_tail(base, last):
        """The last 512 query rows run as two 2-tile sub-groups with an
        interleave of 2 (rows base + 2p + t): after the final EXP only
        the second sub-group's ~2 MB is still on chip, instead of a
        full 4 MB group -- the drain tail halves. Descriptors stay 8 KB
        (one DRAM row per partition)."""
        S = 2
        og = outp.tile([128, S, HW], F16, tag="tail")
        for t in range(S):
            lhs = q_sb[:, base + t:base + 128 * S - S + 1 + t:S]
            fine = last and t == S - 1
            n_sum = 3 if fine else N_ACT
            sums = small.tile([128, n_sum], F32, tag="sums")
            sim_chunk(lhs, og[:, t], 0, ACT_CHUNK, sums[:, 0:1])
            if fine:
                # split the very last chunk so the final exp->
                # normalize->DMA tail is as short as possible
                sim_chunk(lhs, og[:, t], ACT_CHUNK, 1024, sums[:, 1:2])
                sim_chunk(lhs, og[:, t], ACT_CHUNK + 1024, 1024,
                          sums[:, 2:3])
            else:
                sim_chunk(lhs, og[:, t], ACT_CHUNK, ACT_CHUNK,
                          sums[:, 1:2])
            rsum = small.tile([128, 1], F32, tag="rsum")
            nc.vector.tensor_reduce(
                out=rsum, in_=sums, axis=mybir.AxisListType.X,
                op=mybir.AluOpType.add,
            )
            recip = small.tile([128, 1], F32, tag="recip")
            nc.vector.reciprocal(out=recip, in_=rsum)
            lo, hi = base + t, base + 128 * S
            if fine:
                # ship the final tile as two 0.5 MB halves on the two
                # queues in parallel
                for a in range(N_ACT):
                    sl = slice(a * ACT_CHUNK, (a + 1) * ACT_CHUNK)
                    nc.vector.tensor_scalar_mul(
                        out=og[:, t, sl], in0=og[:, t, sl], scalar1=recip
                    )
                    eng = nc.sync if a == 0 else nc.gpsimd
                    eng.dma_start(out=out_ap[lo:hi:S, sl], in_=og[:, t, sl])
            else:
                nc.vector.tensor_scalar_mul(
                    out=og[:, t, :], in0=og[:, t, :], scalar1=recip
                )
                eng = nc.sync if t % 2 == 0 else nc.gpsimd
                eng.dma_start(out=out_ap[lo:hi:S, :], in_=og[:, t, :])

    emit_tail((N_GRP - 1) * 512, last=False)
    emit_tail((N_GRP - 1) * 512 + 256, last=True)


_built = None


def _get_nc():
    global _built
    if _built is None:
        nc = bacc.Bacc("TRN2", target_bir_lowering=False, debug=False)
        x = nc.dram_tensor("x", [C, HW], F16, kind="ExternalInput").ap()
        w = nc.dram_tensor("w", [2 * D, C], F32, kind="ExternalInput").ap()
        out = nc.dram_tensor("out", [HW, HW], F16, kind="ExternalOutput").ap()
        with tile.TileContext(nc) as tc:
            with ExitStack() as ctx:
                _emit(ctx, tc, out, x, w)
        nc.compile()
        _built = nc
    return _built


def kernel(x: np.ndarray, W: np.ndarray) -> np.ndarray:
    nc = _get_nc()
    # x ships as fp16: the kernel previously cast it on-chip before any
    # use, so this is numerically identical and halves the input DMA
    x = np.asarray(x, dtype=np.float32).astype(np.float16)
    W = np.ascontiguousarray(np.asarray(W, dtype=np.float32))
    in_maps = [
        {"x": np.ascontiguousarray(x[b].reshape(C, HW)), "w": W} for b in range(B)
    ]
    res = run_bass_kernel_spmd(nc, in_maps, core_ids=list(range(N_CORES)))
    out = np.stack(
        [res.results[b]["out"].astype(np.float32) for b in range(B)]
    )
    return out[:, None]



# revision 35
# speedup vs baseline: 1.0114x; 1.0037x over previous
"""Spatial self-attention scores kernel for Trainium2 (8 NeuronCores).

Computes, per batch b:
    qk = W @ x_b          # [256, 4096] = [256,256] @ [256,4096]
    q, k = qk[:128], qk[128:]
    sim = (q.T @ k) * 128**-0.5
    out_b = softmax(sim, axis=-1)        # [4096, 4096]
Output: [8, 1, 4096, 4096] float32.

Sharding: data-parallel over batch, one batch image per NeuronCore.

The kernel is ScalarE-bound: softmax's exp runs only on the scalar
engine (1 elem/cycle/lane @ 1.2 GHz => ~109 us body + ~27 us of
instruction/accumulator-read overhead for the 16.7M outputs per
core), so every other phase is arranged to hide under it:
  - x arrives as fp32 via three HWDGE DMAs on the Activation ring
    (ScalarE is idle pre-ramp, and this keeps x off the output ring)
    and is cast fp32->fp16 on DVE in 512-col pieces emitted at their
    point of use (up-front emission head-of-line blocks the in-order
    DVE queue on late x chunks). The ~12 us x transfer time is the
    startup floor.
  - fp16 projection matmuls -> q,k in SBUF as [d=128, s=4096] fp16,
    interleaved with the attention groups; PE warm-up matmuls keep
    the HAM clock ramping while x lands.
  - per 128-query row-tile: fp16 matmuls (K=128, N=512) into 4-bank
    PSUM tiles; one ScalarE ACTIVATE per 2048 columns computes
    exp(SCALE*sim) straight to fp16 with a fused row-sum (accum_out;
    measured cheaper than any DVE-side reduction); DVE combines the
    partial sums, takes the reciprocal, scales the row.
  - the first four row-tiles run column-major (each 512/1536/2048
    wave starts as soon as the x slice it needs has landed), so the
    in-order ACTIVATE queue never waits on the x DMA.
  - query rows interleave across output-group tiles (tile t of group
    g = rows g*512 + 4p + t): each partition then holds 4 consecutive
    DRAM rows = one contiguous 32 KB descriptor per partition per
    4 MB group DMA. Small descriptors otherwise cap the write drain
    at ~250 GB/s (vs ~340 measured with large ones).
  - output leaves as fp16 and is upcast to fp32 on the host (rel-err
    cost ~5e-4 against a 2e-2 budget): halves the HBM write traffic
    that roofline-bound the fp32 version at ~187 us/core; the last
    group ships per normalized half-row to shorten the tail.
"""

import numpy as np
from contextlib import ExitStack

import concourse.bass as bass
import concourse.tile as tile
from concourse import bacc, mybir
from concourse.bass_utils import run_bass_kernel_spmd
from concourse.masks import make_identity

B = 8
C = 256
HW = 4096
D = 128
SCALE = D ** -0.5
N_CORES = 8

BANK = 512             # PSUM bank width (fp32) = one matmul free-dim
ACT_CHUNK = 2048       # one ScalarE activation spans 4 banks
N_ACT = HW // ACT_CHUNK          # 2
GRP = 4                # row-tiles per output DMA (4 -> 4 MB fp16 transfers)
N_GRP = HW // (128 * GRP)        # 8
OUT_BUFS = 3

F32 = mybir.dt.float32
F16 = mybir.dt.float16
MM_DT = mybir.dt.float16
PROJ_DT = mybir.dt.float16

# x input DMA chunks (fp16, HWDGE on the Activation ring). x ships
# from the host already cast to fp16: the kernel used to cast
# fp32->fp16 on DVE before any use, so numerics are identical, but
# the input load halves to 2.1 MB (~6.5 us at the ~330 GB/s per-core
# read path -- splitting across two queues was measured useless, the
# queues just share the same path at half rate each). The first 512
# columns land alone so the first attention wave can start.
X_DMA = ((0, 512), (512, 2048), (2048, 4096))


def _emit(ctx: ExitStack, tc: tile.TileContext, out_ap, x_ap, w_ap):
    nc = tc.nc

    const = ctx.enter_context(tc.tile_pool(name="const", bufs=1))
    data = ctx.enter_context(tc.tile_pool(name="data", bufs=1))
    psum = ctx.enter_context(tc.tile_pool(name="psum", bufs=2, space="PSUM"))
    small = ctx.enter_context(tc.tile_pool(name="small", bufs=4))

    # ---- input DMAs. x (fp16) rides the Activation HWDGE ring:
    # ScalarE is idle until the first ACTIVATE so the trigger cost is
    # free there, and x does not queue behind the output stream on the
    # SP ring. W (tiny, fp32) keeps the SP ring.
    x_view = x_ap.rearrange("(t p) s -> p t s", p=128)
    x_sb = data.tile([128, 2, HW], PROJ_DT)
    nc.scalar.dma_start(
        out=x_sb[:, :, 0:512], in_=x_view[:, :, 0:512]
    )
    w_sb = const.tile([128, 2, C], F32)
    nc.sync.dma_start(out=w_sb, in_=w_ap.rearrange("(t p) c -> p t c", p=128))
    for lo, hi in X_DMA[1:]:
        nc.scalar.dma_start(out=x_sb[:, :, lo:hi], in_=x_view[:, :, lo:hi])

    # ---- PE warm-up: throwaway matmuls while x is loading. The PE
    # clock (HAM) only ramps after sustained activity; warming during
    # the input DMA makes the projection and the first attention
    # row-tiles run at full rate.
    warm_f32 = const.tile([128, BANK], F32)
    nc.vector.memset(warm_f32, 0.0)
    warm = const.tile([128, BANK], MM_DT)
    nc.vector.tensor_copy(out=warm, in_=warm_f32)
    wps = psum.tile([128, ACT_CHUNK], F32, tag="ps")
    for _ in range(4):
        nc.tensor.matmul(
            wps[:, 0:BANK], warm[:, 0:128], warm, start=True, stop=True
        )

    ident = const.tile([128, 128], F32)
    make_identity(nc, ident)

    # pull the exp table load off the first real activation; the dummy
    # accum_out read resets the ACT accumulator register so the warm-up
    # exp(0)=1 does not leak into the first row's sum
    tbl = small.tile([128, 2], F32, tag="tbl")
    nc.scalar.activation(
        out=tbl[:, 0:1], in_=warm_f32[:, 0:1],
        func=mybir.ActivationFunctionType.Exp, accum_out=tbl[:, 1:2],
    )

    # ---- transpose W on PE -> wt_sb[c_sub, c_tile, o] (contraction c on partitions)
    wt_sb = const.tile([128, 2, 2 * D], PROJ_DT)
    for t in range(2):          # output-channel tile (q half / k half)
        for ct in range(2):     # input-channel tile
            ps = psum.tile([128, ACT_CHUNK], F32, tag="ps")
            nc.tensor.transpose(
                ps[:, 0:128], w_sb[:, t, ct * 128:(ct + 1) * 128], ident
            )
            nc.vector.tensor_copy(
                out=wt_sb[:, ct, t * 128:(t + 1) * 128], in_=ps[:, 0:128]
            )
    # keep the PE clock ramping while the first x chunk lands (kept
    # short: these sit in PE program order ahead of the projections)
    wps2 = psum.tile([128, ACT_CHUNK], F32, tag="ps")
    for _ in range(2):
        nc.tensor.matmul(
            wps2[:, 0:BANK], warm[:, 0:128], warm, start=True, stop=True
        )

    q_sb = data.tile([128, HW], MM_DT)
    k_sb = data.tile([128, HW], MM_DT)

    def proj_cols(t, dst, lo, hi):
        """Project output-channel half t (0=q, 1=k) for columns [lo, hi)
        (hi-lo <= 2048) in <=512-wide bank pieces."""
        ps = psum.tile([128, ACT_CHUNK], F32, tag="ps")
        for j in range(0, hi - lo, BANK):
            n = min(BANK, hi - lo - j)
            sl = slice(lo + j, lo + j + n)
            psl = slice(j, j + n)
            for ct in range(2):
                nc.tensor.matmul(
                    ps[:, psl], wt_sb[:, ct, t * 128:(t + 1) * 128],
                    x_sb[:, ct, sl], start=(ct == 0), stop=(ct == 1),
                )
            nc.vector.tensor_copy(out=dst[:, sl], in_=ps[:, psl])

    outp = None
    # Query-row interleave: tile t of group g covers query rows
    # g*256 + 2p + t (p = partition). Per partition the two tiles are
    # CONSECUTIVE DRAM rows -> one contiguous 16 KB descriptor per
    # partition in the group DMA. 8 KB descriptors (row-per-partition)
    # cap the write drain at ~250 GB/s; 16 KB reach ~358 GB/s.
    out_view = out_ap.rearrange("(g p t) m -> g p t m", t=GRP, p=128)

    def tile_lhs(g, t):
        lo = g * GRP * 128 + t
        return q_sb[:, lo:lo + (GRP * 128 - GRP + 1):GRP]

    def sim_chunk(lhs, out_row, lo_col, n_col, accum):
        """n_col-wide slice of one attention row: matmuls + fused exp.

        Row sums come from the ACTIVATE's per-instruction accumulator
        (it does NOT persist across ACTIVATEs -- measured), costing a
        ~290ns ACTIVATION_READ_ACCUMULATOR on ScalarE per chunk. All
        DVE-side alternatives were MEASURED slower: tensor_reduce has
        no perf mode (2.13us/chunk, +11us end to end) and the
        tensor_scalar+accum_out variant, despite the IR model claiming
        4x_2p eligibility, runs at 1x on hardware (2.29us/chunk; the
        reduce tree takes one element per cycle) -- pushing DVE to 85%
        busy and +60us end to end. The accumulator read is genuinely
        the cheapest summer."""
        ps = psum.tile([128, ACT_CHUNK], F32, tag="ps")
        for jj in range(0, n_col, BANK):
            n = min(BANK, n_col - jj)
            sl = slice(lo_col + jj, lo_col + jj + n)
            nc.tensor.matmul(
                ps[:, jj:jj + n], lhs, k_sb[:, sl],
                start=True, stop=True,
            )
        sl = slice(lo_col, lo_col + n_col)
        nc.scalar.activation(
            out=out_row[:, sl],
            in_=ps[:, 0:n_col],
            func=mybir.ActivationFunctionType.Exp,
            scale=SCALE,
            accum_out=accum,
        )

    def normalize_tile(out_grp, g, t, rsum, split_dma):
        recip = small.tile([128, 1], F32, tag="recip")
        nc.vector.reciprocal(out=recip, in_=rsum)
        lo = g * GRP * 128 + t
        hi = lo + GRP * 128 - GRP + 1
        if split_dma == "half":
            # normalize and ship each half-row as soon as it is scaled
            # (0.5 MB transfers, 4 KB descriptors): last group only,
            # for the shortest possible tail. The two halves go to
            # different queues (SP + GpSimd SWDGE) so they drain in
            # parallel.
            for a in range(N_ACT):
                sl = slice(a * ACT_CHUNK, (a + 1) * ACT_CHUNK)
                nc.vector.tensor_scalar_mul(
                    out=out_grp[:, t, sl], in0=out_grp[:, t, sl],
                    scalar1=recip,
                )
                eng = nc.sync if a == 0 else nc.gpsimd
                eng.dma_start(
                    out=out_ap[lo:hi:GRP, sl],
                    in_=out_grp[:, t, sl],
                )
        else:
            nc.vector.tensor_scalar_mul(
                out=out_grp[:, t, :], in0=out_grp[:, t, :], scalar1=recip
            )
            if split_dma == "early":
                # The first group's tiles leave via GpSimd SWDGE: the
                # engine is otherwise idle, SWDGE uses separate queue
                # slots, and these 4 MB have ~100us of slack. Keeping
                # them out of the SP-ring FIFO means the output queue
                # never builds the ramp-time backlog that otherwise
                # takes ~13us to drain after the last ACTIVATE.
                nc.gpsimd.dma_start(
                    out=out_ap[lo:hi:GRP, :], in_=out_grp[:, t, :]
                )
            elif split_dma == "tile":
                # per-tile 1 MB HWDGE DMAs (8 KB descriptors)
                nc.sync.dma_start(
                    out=out_ap[lo:hi:GRP, :], in_=out_grp[:, t, :]
                )
            elif split_dma == "tile2q":
                # tail groups: alternate tiles between the SP ring and
                # the GpSimd SWDGE queue so the final ~8 MB drains on
                # two queues in parallel (one queue takes ~11us for
                # the last group alone, all of it after the last EXP)
                eng = nc.sync if t % 2 == 0 else nc.gpsimd
                eng.dma_start(
                    out=out_ap[lo:hi:GRP, :], in_=out_grp[:, t, :]
                )
            elif split_dma == "tile3q":
                # last group: three queues. Tile 0 rides the Activation
                # HWDGE ring -- idle since the x load -- with the
                # trigger emitted right here so ScalarE reaches it
                # during tile 1's EXPs (its normalize-mul wait already
                # satisfied; emitted any later it would head-of-line
                # block the remaining EXPs in the in-order queue).
                eng = (nc.scalar, nc.gpsimd, nc.sync)[t % 3]
                eng.dma_start(
                    out=out_ap[lo:hi:GRP, :], in_=out_grp[:, t, :]
                )

    def emit_group(g, split_dma=None, fine_tail=False, mid=None):
        out_grp = outp.tile([128, GRP, HW], F16, tag="out")
        for t in range(GRP):
            if t == 2 and mid is not None:
                # next group's q projection, emitted mid-group: by the
                # time group g+1's first sim matmul needs it, the PE
                # matmuls and DVE copies have long cleared -- issued
                # at the group boundary they cost a ~500ns ScalarE gap
                # per group (q-copy latency on the in-order DVE queue)
                mid()
            lhs = tile_lhs(g, t)
            fine = fine_tail and t == GRP - 1
            n_sum = 3 if fine else N_ACT
            sums = small.tile([128, n_sum], F32, tag="sums")
            sim_chunk(lhs, out_grp[:, t], 0, ACT_CHUNK, sums[:, 0:1])
            if fine:
                # split the very last chunk so the final exp->normalize->
                # DMA tail is as short as possible
                sim_chunk(lhs, out_grp[:, t], ACT_CHUNK, 1024,
                          sums[:, 1:2])
                sim_chunk(lhs, out_grp[:, t], ACT_CHUNK + 1024, 1024,
                          sums[:, 2:3])
            else:
                sim_chunk(lhs, out_grp[:, t], ACT_CHUNK, ACT_CHUNK,
                          sums[:, 1:2])
            rsum = small.tile([128, 1], F32, tag="rsum")
            nc.vector.tensor_reduce(
                out=rsum, in_=sums, axis=mybir.AxisListType.X,
                op=mybir.AluOpType.add,
            )
            # the very last tile ships as two pipelined 0.5 MB halves
            # so the final bytes leave ~1.5us sooner than one 1 MB DMA
            normalize_tile(out_grp, g, t, rsum,
                           "half" if fine else split_dma)
        if not split_dma:
            nc.sync.dma_start(out=out_view[g], in_=out_grp)

    def emit_early_groups():
        """Groups 0-1, reordered chunk-major: all four row-tiles' lower
        (cols 0:2048) chunks run first -- they only need the first half
        of x -- bridging ScalarE across the ~18 us it takes the upper
        half of x to arrive; the upper chunks and the normalizes follow.
        The very first row-tile runs 512/512/1024-wide so the first
        ACTIVATE fires as soon as the first 512 columns of x land."""
        og = outp.tile([128, GRP, HW], F16, tag="out", name="og0")
        sums = [small.tile([128, 3], F32, tag="sums", name=f"esums{i}")
                for i in range(4)]
        lhs = [tile_lhs(0, i) for i in range(4)]
        rows = [og[:, i] for i in range(4)]
        # column-major across the four tiles, 512/1536/2048 waves. The
        # PE runs at ~0.81 GHz until t~20us (fixed DVFS ramp), so the
        # early phase is PE-supply-bound; group 0's EXPs end ~35us in
        # every arrangement tried. This one (512-wide c0 ASAP, the
        # projection train after it) measured best: a 1024-wide c0
        # delayed the first EXP ~1.5us for no downstream gain, the
        # train emitted before c0 pushed it 7.5us later, and finer
        # 512-wide wave interleaving cost ~3us of extra ACTIVATE
        # overhead and wave jitter.
        for i in range(4):
            sim_chunk(lhs[i], rows[i], 0, 512, sums[i][:, 0:1])
        proj_cols(1, k_sb, 512, 2048)
        proj_cols(1, k_sb, 2048, 3072)
        proj_cols(1, k_sb, 3072, 4096)
        proj_cols(0, q_sb, 512, 1024)
        for i in range(4):
            sim_chunk(lhs[i], rows[i], 512, 1536, sums[i][:, 1:2])
        for i in range(4):
            sim_chunk(lhs[i], rows[i], 2048, 2048, sums[i][:, 2:3])
            rsum = small.tile([128, 1], F32, tag="rsum")
            nc.vector.tensor_reduce(
                out=rsum, in_=sums[i], axis=mybir.AxisListType.X,
                op=mybir.AluOpType.add,
            )
            normalize_tile(og, 0, i, rsum, "early")

    # ---- projection, interleaved with the attention groups so the
    # in-order PE reaches the first ACTIVATE as early as possible.
    proj_cols(1, k_sb, 0, 512)      # k cols 0:512 (first x chunk)
    proj_cols(0, q_sb, 0, 512)      # q rows 0:512 -> groups 0-1

    outp = ctx.enter_context(tc.tile_pool(name="outp", bufs=OUT_BUFS))
    emit_early_groups()
    # group g consumes q columns [g*512, (g+1)*512); each group's q
    # window is projected during the PREVIOUS group (mid=), a full
    # group ahead of its first consumer, so the group-boundary sim
    # matmul never waits on the q PSUM->SBUF copy.
    # Ship plan: the single SP-ring queue bursts at ~390 GB/s but a
    # whole 4 MB group bunches right at that group's normalize, so
    # group 5 rides the otherwise-idle GpSimd SWDGE queue and the two
    # tail groups alternate tiles across both queues -- the final ~8 MB
    # drains in parallel instead of serially after the last EXP.
    # group 1's q window was already projected inside the early phase
    ship = {1: "tile", 5: "early", 6: "tile2q"}
    for g in range(1, N_GRP - 1):
        def mid(gg=g):
            proj_cols(0, q_sb, (gg + 1) * 512, (gg + 2) * 512)
        emit_group(g, split_dma=ship.get(g), mid=mid)

    def emit_tail(base, last):
        """The last 512 query rows run as two 2-tile sub-groups with an
        interleave of 2 (rows base + 2p + t): after the final EXP only
        the second sub-group's ~2 MB is still on chip, instead of a
        full 4 MB group -- the drain tail halves. Descriptors stay 8 KB
        (one DRAM row per partition)."""
        S = 2
        og = outp.tile([128, S, HW], F16, tag="tail")
        for t in range(S):
            lhs = q_sb[:, base + t:base + 128 * S - S + 1 + t:S]
            fine = last and t == S - 1
            n_sum = 3 if fine else N_ACT
            sums = small.tile([128, n_sum], F32, tag="sums")
            sim_chunk(lhs, og[:, t], 0, ACT_CHUNK, sums[:, 0:1])
            if fine:
                # split the very last chunk so the final exp->
                # normalize->DMA tail is as short as possible
                sim_chunk(lhs, og[:, t], ACT_CHUNK, 1024, sums[:, 1:2])
                sim_chunk(lhs, og[:, t], ACT_CHUNK + 1024, 1024,
                          sums[:, 2:3])
            else:
                sim_chunk(lhs, og[:, t], ACT_CHUNK, ACT_CHUNK,
                          sums[:, 1:2])
            rsum = small.tile([128, 1], F32, tag="rsum")
            nc.vector.tensor_reduce(
                out=rsum, in_=sums, axis=mybir.AxisListType.X,
                op=mybir.AluOpType.add,
            )
            recip = small.tile([128, 1], F32, tag="recip")
            nc.vector.reciprocal(out=recip, in_=rsum)
            lo, hi = base + t, base + 128 * S
            if fine:
                # ship the final tile as two 0.5 MB halves on the two
                # queues in parallel
                for a in range(N_ACT):
                    sl = slice(a * ACT_CHUNK, (a + 1) * ACT_CHUNK)
                    nc.vector.tensor_scalar_mul(
                        out=og[:, t, sl], in0=og[:, t, sl], scalar1=recip
                    )
                    eng = nc.sync if a == 0 else nc.gpsimd
                    eng.dma_start(out=out_ap[lo:hi:S, sl], in_=og[:, t, sl])
            else:
                nc.vector.tensor_scalar_mul(
                    out=og[:, t, :], in0=og[:, t, :], scalar1=recip
                )
                eng = nc.sync if t % 2 == 0 else nc.gpsimd
                eng.dma_start(out=out_ap[lo:hi:S, :], in_=og[:, t, :])

    emit_tail((N_GRP - 1) * 512, last=False)
    emit_tail((N_GRP - 1) * 512 + 256, last=True)


_built = None


def _get_nc():
    global _built
    if _built is None:
        nc = bacc.Bacc("TRN2", target_bir_lowering=False, debug=False)
        x = nc.dram_tensor("x", [C, HW], F16, kind="ExternalInput").ap()
        w = nc.dram_tensor("w", [2 * D, C], F32, kind="ExternalInput").ap()
        out = nc.dram_tensor("out", [HW, HW], F16, kind="ExternalOutput").ap()
        with tile.TileContext(nc) as tc:
            with ExitStack() as ctx:
                _emit(ctx, tc, out, x, w)
        nc.compile()
        _built = nc
    return _built


def kernel(x: np.ndarray, W: np.ndarray) -> np.ndarray:
    nc = _get_nc()
    # x ships as fp16: the kernel previously cast it on-chip before any
    # use, so this is numerically identical and halves the input DMA
    x = np.asarray(x, dtype=np.float32).astype(np.float16)
    W = np.ascontiguousarray(np.asarray(W, dtype=np.float32))
    in_maps = [
        {"x": np.ascontiguousarray(x[b].reshape(C, HW)), "w": W} for b in range(B)
    ]
    res = run_bass_kernel_spmd(nc, in_maps, core_ids=list(range(N_CORES)))
    out = np.stack(
        [res.results[b]["out"].astype(np.float32) for b in range(B)]
    )
    return out[:, None]

